# revision 1
# baseline (speedup 1.0000x reference)
"""Trainium2 Bass kernel for a 2-layer CIN (Compressed Interaction Network).

Reference computation (per batch b, embedding dim d):
    h1[h] = sum_{f,q} x[f] x[q] W0[h, f*39+q]          (f,q in 0..38)
    h2[h] = sum_{f,q} x[f] h1[q] W1[h, f*128+q]        (f in 0..38, q in 0..127)
    out[b] = concat(sum_d h1, sum_d h2)                 -> [B, 256]

Device mapping (data-parallel over batch across 8 cores, 256 b's each):
  * Layer 1 uses a polarization ("sum of squares") identity so the outer
    product x (x) x never materializes:  x_i x_j = ((x_i+x_j)^2 - x_i^2 - x_j^2)/2.
    With 780 fixed linear forms V (39 singles + 741 pair-sums) and a
    re-packed coefficient matrix C:  h1 = C^T (V^T x)^2.
  * Layer 2 materializes z1[f-block] = broadcast(x_f) * h1 per f via a
    one-hot "selector" matmul broadcast (PE) + elementwise multiply (DVE),
    then accumulates W1-block matmuls into PSUM.
  * Columns are (b, d) pairs, d innermost; per-core tiles of 32 b = 512 cols.
"""

import numpy as np

import concourse.mybir as mybir
import concourse.tile as tile
from concourse import bacc
from concourse.bass import ts
from concourse.bass_utils import run_bass_kernel_spmd

B, F0, D = 2048, 39, 16
H1, H2 = 128, 128
NCORES = 8
BC = B // NCORES          # 256 batches per core
BT = 32                   # batches per tile
NT = BC // BT             # 8 tiles per core
N = BT * D                # 512 columns per tile
NPAIR = F0 * (F0 - 1) // 2  # 741
NFORM = F0 + NPAIR        # 780
NFP = 784                 # padded to 7 * 112
NCHUNK = 7
CW = 112                  # forms per chunk

F16 = mybir.dt.float16
F32 = mybir.dt.float32


def pack_weights(W0: np.ndarray, W1: np.ndarray):
    """Host-side repack of CIN weights into device layouts (fp16)."""
    W0m = W0[:, :, 0].reshape(H1, F0, F0).astype(np.float64)
    W1m = W1[:, :, 0].reshape(H2, F0, H1).astype(np.float64)

    V = np.zeros((F0, NFP), dtype=np.float64)
    C = np.zeros((NFP, H1), dtype=np.float64)
    for i in range(F0):
        V[i, i] = 1.0
        Bi = W0m[:, i, :] + W0m[:, :, i]          # [H, F]
        C[i, :] = W0m[:, i, i] - 0.5 * (Bi.sum(axis=1) - 2.0 * W0m[:, i, i])
    k = F0
    for i in range(F0):
        for j in range(i + 1, F0):
            V[i, k] = 1.0
            V[j, k] = 1.0
            C[k, :] = 0.5 * (W0m[:, i, j] + W0m[:, j, i])
            k += 1
    # chunk layouts: V [39, 784]; C -> [112, 7, 128]
    c_pack = C.reshape(NCHUNK, CW, H1).transpose(1, 0, 2)

    selb = np.zeros((F0, F0 * H1), dtype=np.float64)
    for f in range(F0):
        selb[f, f * H1:(f + 1) * H1] = 1.0

    w1p = W1m.transpose(2, 1, 0)                   # [q=128, f=39, h=128]

    return {
        "vp": np.ascontiguousarray(V, dtype=np.float16),
        "cp": np.ascontiguousarray(c_pack, dtype=np.float16),
        "selb": np.ascontiguousarray(selb, dtype=np.float16),
        "w1p": np.ascontiguousarray(w1p, dtype=np.float16),
    }


def build(reps: int = 1):
    """Build the per-core Bass module. reps>1 wraps the body in a HW loop
    (for wall-clock timing only — the graded path uses reps=1)."""
    nc = bacc.Bacc("TRN2", target_bir_lowering=False, debug=False,
                   num_devices=NCORES)

    x_h = nc.dram_tensor("x", [BC, F0, D], F16, kind="ExternalInput")
    vp_h = nc.dram_tensor("vp", [F0, NFP], F16, kind="ExternalInput")
    cp_h = nc.dram_tensor("cp", [CW, NCHUNK, H1], F16, kind="ExternalInput")
    selb_h = nc.dram_tensor("selb", [F0, F0 * H1], F16, kind="ExternalInput")
    w1p_h = nc.dram_tensor("w1p", [H1, F0, H2], F16, kind="ExternalInput")
    out_h = nc.dram_tensor("out", [2, 128, BC], F32, kind="ExternalOutput")

    x_ap = x_h.ap().rearrange("b f d -> f b d")    # [39, 256, 16]

    with tile.TileContext(nc) as tc:
        with (
            tc.tile_pool(name="const", bufs=1) as const,
            tc.tile_pool(name="xp", bufs=2) as xp,
            tc.tile_pool(name="ysq", bufs=2) as ysqp,
            tc.tile_pool(name="hsb", bufs=2) as hsbp,
            tc.tile_pool(name="bc", bufs=3) as bcp,
            tc.tile_pool(name="z", bufs=3) as zp,
            tc.tile_pool(name="yps", bufs=2, space="PSUM") as yps,
            tc.tile_pool(name="h1ps", bufs=2, space="PSUM") as h1psp,
            tc.tile_pool(name="h2ps", bufs=2, space="PSUM") as h2psp,
            tc.tile_pool(name="bcps", bufs=2, space="PSUM") as bcps,
        ):
            v_sb = const.tile([F0, NFP], F16)
            nc.sync.dma_start(out=v_sb[:], in_=vp_h.ap())
            c_sb = const.tile([CW, NCHUNK, H1], F16)
            nc.sync.dma_start(out=c_sb[:], in_=cp_h.ap())
            sel_sb = const.tile([F0, F0, H1], F16)
            nc.sync.dma_start(out=sel_sb[:], in_=selb_h.ap())
            w1_sb = const.tile([H1, F0, H2], F16)
            nc.sync.dma_start(out=w1_sb[:], in_=w1p_h.ap())
            out1_sb = const.tile([128, BC], F32)
            out2_sb = const.tile([128, BC], F32)

            def body(_i=None):
                for t in range(NT):
                    x_sb = xp.tile([F0, BT, D], F16)
                    nc.sync.dma_start(out=x_sb[:], in_=x_ap[:, ts(t, BT), :])
                    x_flat = x_sb[:, :, :]  # free = 512

                    # ---- layer 1: h1 = C^T (V^T x)^2 ----
                    ysq = ysqp.tile([CW, NCHUNK, N], F16)
                    for j in range(NCHUNK):
                        y_ps = yps.tile([CW, N], F32)
                        nc.tensor.matmul(y_ps[:], v_sb[:, ts(j, CW)], x_flat,
                                         start=True, stop=True)
                        nc.scalar.square(ysq[:, j, :], y_ps[:])
                    h1_ps = h1psp.tile([H1, N], F32)
                    for j in range(NCHUNK):
                        nc.tensor.matmul(h1_ps[:], c_sb[:, j, :], ysq[:, j, :],
                                         start=(j == 0), stop=(j == NCHUNK - 1))
                    h1_sb = hsbp.tile([H1, N], F16)
                    nc.scalar.copy(h1_sb[:], h1_ps[:])
                    nc.vector.reduce_sum(
                        out=out1_sb[:, ts(t, BT)],
                        in_=h1_ps.rearrange("p (b d) -> p b d", d=D),
                        axis=mybir.AxisListType.X,
                    )

                    # ---- layer 2: h2 = sum_f W1[:,f,:] @ (h1 * bcast(x_f)) ----
                    h2_ps = h2psp.tile([H2, N], F32)
                    for f in range(F0):
                        bc_ps = bcps.tile([H1, N], F32)
                        nc.tensor.matmul(bc_ps[:], sel_sb[:, f, :], x_flat,
                                         start=True, stop=True)
                        bc_sb = bcp.tile([H1, N], F16)
                        nc.scalar.copy(bc_sb[:], bc_ps[:])
                        z_sb = zp.tile([H1, N], F16)
                        nc.vector.tensor_mul(out=z_sb[:], in0=h1_sb[:],
                                             in1=bc_sb[:])
                        nc.tensor.matmul(h2_ps[:], w1_sb[:, f, :], z_sb[:],
                                         start=(f == 0), stop=(f == F0 - 1))
                    nc.vector.reduce_sum(
                        out=out2_sb[:, ts(t, BT)],
                        in_=h2_ps.rearrange("p (b d) -> p b d", d=D),
                        axis=mybir.AxisListType.X,
                    )

                nc.sync.dma_start(out=out_h.ap()[0], in_=out1_sb[:])
                nc.sync.dma_start(out=out_h.ap()[1], in_=out2_sb[:])

            if reps == 1:
                body()
            else:
                with tc.For_i(0, reps) as i:
                    body(i)

    nc.compile()
    return nc


_CACHE: dict = {}


def _get_module(reps: int = 1):
    if reps not in _CACHE:
        _CACHE[reps] = build(reps)
    return _CACHE[reps]


def run(input: np.ndarray, W0: np.ndarray, W1: np.ndarray, reps: int = 1):
    nc = _get_module(reps)
    packs = pack_weights(np.asarray(W0), np.asarray(W1))
    x16 = np.ascontiguousarray(np.asarray(input), dtype=np.float16)
    in_maps = []
    for c in range(NCORES):
        m = {"x": x16[c * BC:(c + 1) * BC]}
        m.update(packs)
        in_maps.append(m)
    res = run_bass_kernel_spmd(nc, in_maps, core_ids=list(range(NCORES)))
    out = np.empty((B, 256), dtype=np.float32)
    for c in range(NCORES):
        o = res.results[c]["out"]          # [2, 128, 256]
        out[c * BC:(c + 1) * BC, :128] = o[0].T
        out[c * BC:(c + 1) * BC, 128:] = o[1].T
    return out


def kernel(input: np.ndarray, W0: np.ndarray, W1: np.ndarray) -> np.ndarray:
    return run(input, W0, W1, reps=1)


# revision 11
# speedup vs baseline: 2.0496x; 2.0496x over previous
"""Trainium2 Bass kernel for a 2-layer CIN (Compressed Interaction Network).

Reference computation (per batch b, embedding dim d):
    h1[q] = sum_{f,g} x[f] x[g] W0[q, f*39+g]          (f,g in 0..38)
    h2[h] = sum_{f,q} x[f] h1[q] W1[h, f*128+q]        (f in 0..38, q in 0..127)
    out[b] = concat(sum_d h1, sum_d h2)                 -> [B, 256]

Device mapping (data-parallel over batch across 8 cores, 256 b's each):
  * Layer 1 uses a polarization ("sum of squares") identity so the outer
    product x (x) x never materializes:  x_i x_j = ((x_i+x_j)^2 - x_i^2 - x_j^2)/2.
    With 780 fixed linear forms V (39 singles + 741 pair sums, padded to
    896 = 7*128) and re-packed coefficients C:  h1 = C^T (V^T x)^2.
    All matmuls padded to contraction k=128 (k<128 measured 3.7x slower).
  * Layer 2 exploits  sum_d h2[b,:,d] = W1flat @ vec(S_b),
    S_b[f,q] = sum_d x[b,f,d] h1[b,q,d]  (a k=16 outer-product contraction
    per batch).  S^T is computed 3 batches at a time with one k=128 matmul
    against a host-precomputed block-diagonal transposed-x operand, after
    transposing h1 on the PE.  The final contraction is 39 k=128 matmuls.
"""

import numpy as np

import concourse.mybir as mybir
import concourse.tile as tile
from concourse import bacc
from concourse.bass import ts
from concourse.bass_utils import run_bass_kernel_spmd

B, F0, D = 2048, 39, 16
H1, H2 = 128, 128
NCORES = 8
BC = B // NCORES          # 256 batches per core
BT = 32                   # batches per tile
NT = BC // BT             # 8 tiles per core
N = BT * D                # 512 columns per tile (cols = (b, d), d inner)
NFP = 896                 # forms padded to 7*128
NCHUNK = 7
CW = 128                  # forms per chunk
GB = 3                    # batches per S-chunk (3*39=117 <= 128)
NG = 11                   # S-chunks per tile (10*3 + 1*2 = 32)
DP = 32                   # padded d-block (16 real + 16 zero)

F16 = mybir.dt.float16
F32 = mybir.dt.float32


def pack_weights(W0: np.ndarray, W1: np.ndarray):
    """Host-side repack of CIN weights into device layouts (fp16)."""
    W0m = W0[:, :, 0].reshape(H1, F0, F0).astype(np.float64)
    W1m = W1[:, :, 0].reshape(H2, F0, H1).astype(np.float64)

    V = np.zeros((128, NFP), dtype=np.float64)   # k-padded: rows 39.. = 0
    C = np.zeros((NFP, H1), dtype=np.float64)
    for i in range(F0):
        V[i, i] = 1.0
        Bi = W0m[:, i, :] + W0m[:, :, i]          # [H, F]
        C[i, :] = W0m[:, i, i] - 0.5 * (Bi.sum(axis=1) - 2.0 * W0m[:, i, i])
    k = F0
    for i in range(F0):
        for j in range(i + 1, F0):
            V[i, k] = 1.0
            V[j, k] = 1.0
            C[k, :] = 0.5 * (W0m[:, i, j] + W0m[:, j, i])
            k += 1
    c_pack = C.reshape(NCHUNK, CW, H1).transpose(1, 0, 2)   # [128, 7, 128]

    w1p = W1m.transpose(2, 1, 0)                   # [q=128, f=39, h=128]

    ident = np.eye(128, dtype=np.float16)

    return {
        "vp": np.ascontiguousarray(V, dtype=np.float16),
        "cp": np.ascontiguousarray(c_pack, dtype=np.float16),
        "w1p": np.ascontiguousarray(w1p, dtype=np.float16),
        "ident": ident,
    }


def pack_x(x_core: np.ndarray):
    """Per-core input repack: f-padded dense x + block-diagonal transposed x.

    x_core: [BC, 39, 16] float.
    Returns xp [BC, 128, 16] fp16 (f rows 39.. zero) and
    xt3 [NT, NG, 128, 117] fp16: chunk (t,c) covers batches 32t+3c+j,
    partition p=(j*32+d), col=(j*39+f), value x[b, f, d] (zero-padded).
    """
    x16 = x_core.astype(np.float16)
    xp = np.zeros((BC, 128, D), dtype=np.float16)
    xp[:, :F0, :] = x16
    xt3 = np.zeros((NT, NG, 128, GB * F0), dtype=np.float16)
    for t in range(NT):
        for c in range(NG):
            for j in range(GB):
                b = 32 * t + 3 * c + j
                if b >= 32 * (t + 1):
                    continue
                # block j: partitions j*32 + d (d<16), cols j*39 + f
                xt3[t, c, j * DP:j * DP + D, j * F0:(j + 1) * F0] = (
                    x16[b].T)  # [16 d, 39 f]
    return xp, xt3


def build(reps: int = 1):
    """Build the per-core Bass module. reps>1 wraps the body in a HW loop
    (wall-clock timing only — the graded path uses reps=1)."""
    nc = bacc.Bacc("TRN2", target_bir_lowering=False, debug=False,
                   num_devices=NCORES)

    x_h = nc.dram_tensor("xp", [BC, 128, D], F16, kind="ExternalInput")
    xt3_h = nc.dram_tensor("xt3", [NT, NG, 128, GB * F0], F16,
                           kind="ExternalInput")
    vp_h = nc.dram_tensor("vp", [128, NFP], F16, kind="ExternalInput")
    cp_h = nc.dram_tensor("cp", [CW, NCHUNK, H1], F16, kind="ExternalInput")
    w1p_h = nc.dram_tensor("w1p", [H1, F0, H2], F16, kind="ExternalInput")
    id_h = nc.dram_tensor("ident", [128, 128], F16, kind="ExternalInput")
    out_h = nc.dram_tensor("out", [2, 128, BC], F32, kind="ExternalOutput")

    x_ap = x_h.ap().rearrange("b f d -> f b d")      # [128, 256, 16]
    xt3_ap = xt3_h.ap().rearrange("t c p w -> p t c w")  # [128, 8, 11, 117]

    with tile.TileContext(nc) as tc:
        with (
            tc.tile_pool(name="const", bufs=1) as const,
            tc.tile_pool(name="xpool", bufs=2) as xpool,
            tc.tile_pool(name="xtp", bufs=2) as xtp,
            tc.tile_pool(name="ysq", bufs=2) as ysqp,
            tc.tile_pool(name="h1p3", bufs=2) as h1p3p,
            tc.tile_pool(name="h1t", bufs=2) as h1tp,
            tc.tile_pool(name="yps", bufs=2, space="PSUM") as yps,
            tc.tile_pool(name="h1ps", bufs=2, space="PSUM") as h1psp,
            tc.tile_pool(name="h1tps", bufs=2, space="PSUM") as h1tps,
            tc.tile_pool(name="stps", bufs=2, space="PSUM") as stps,
        ):
            v_sb = const.tile([128, NFP], F16)
            nc.sync.dma_start(out=v_sb[:], in_=vp_h.ap())
            c_sb = const.tile([CW, NCHUNK, H1], F16)
            nc.sync.dma_start(out=c_sb[:], in_=cp_h.ap())
            w1_sb = const.tile([H1, F0, H2], F16)
            nc.sync.dma_start(out=w1_sb[:], in_=w1p_h.ap())
            id_sb = const.tile([128, 128], F16)
            nc.sync.dma_start(out=id_sb[:], in_=id_h.ap())
            out1_sb = const.tile([128, BC], F32)
            sall_sb = const.tile([128, BC, F0], F16)   # S^T: [q, b, f]

            def body(_i=None):
                for t in range(NT):
                    x_sb = xpool.tile([128, BT, D], F16)
                    nc.sync.dma_start(out=x_sb[:], in_=x_ap[:, ts(t, BT), :])
                    x_flat = x_sb[:, :, :]           # [128, 512]
                    xt_sb = xtp.tile([128, NG, GB * F0], F16)
                    nc.sync.dma_start(out=xt_sb[:], in_=xt3_ap[:, t])

                    # ---- layer 1: h1 = C^T (V^T x)^2, all k=128 ----
                    ysq = ysqp.tile([128, NCHUNK, N], F16)
                    for j in range(NCHUNK):
                        y_ps = yps.tile([128, N], F32, tag="y")
                        nc.tensor.matmul(y_ps[:], v_sb[:, ts(j, CW)], x_flat,
                                         start=True, stop=True)
                        nc.scalar.square(ysq[:, j, :], y_ps[:])
                    h1_ps = h1psp.tile([H1, N], F32)
                    for j in range(NCHUNK):
                        nc.tensor.matmul(h1_ps[:], c_sb[:, j, :], ysq[:, j, :],
                                         start=(j == 0), stop=(j == NCHUNK - 1))
                    nc.vector.reduce_sum(
                        out=out1_sb[:, ts(t, BT)],
                        in_=h1_ps.rearrange("p (b d) -> p b d", d=D),
                        axis=mybir.AxisListType.X,
                    )

                    # ---- h1 -> padded (3b x 32d + 32z) column blocks ----
                    h1p3 = h1p3p.tile([128, NG, 128], F16)
                    nc.gpsimd.memset(h1p3[:], 0.0)
                    # c = 0..9 (3 batches each), c = 10 (2 batches)
                    nc.scalar.copy(
                        h1p3[:, :10, :3 * DP].rearrange(
                            "p c (j dp) -> p c j dp", dp=DP)[:, :, :, :D],
                        h1_ps[:, :480].rearrange("p (c j d) -> p c j d",
                                                 j=GB, d=D),
                    )
                    nc.scalar.copy(
                        h1p3[:, 10, :2 * DP].rearrange(
                            "p (j dp) -> p j dp", dp=DP)[:, :, :D],
                        h1_ps[:, 480:].rearrange("p (j d) -> p j d", d=D),
                    )

                    # ---- transpose h1 blocks; S^T-matmuls (k=128) ----
                    h1t_sb = h1tp.tile([128, NG, 128], F16)
                    for half in range(2):            # chunks 0-5, 6-10
                        c0 = 6 * half
                        ncnk = 6 if half == 0 else 5
                        h1t_ps = h1tps.tile([128, 6, 128], F16, tag="h1t")
                        for u in range(ncnk):
                            nc.tensor.transpose(h1t_ps[:, u, :],
                                                h1p3[:, c0 + u, :], id_sb[:])
                        nc.vector.tensor_copy(
                            out=h1t_sb[:, c0:c0 + ncnk, :],
                            in_=h1t_ps[:, :ncnk, :])

                    for grp in range(3):             # S chunks 0-3, 4-7, 8-10
                        g0 = 4 * grp
                        ncnk = 4 if grp < 2 else 3
                        st_ps = stps.tile([128, 4, GB * F0], F32, tag="st")
                        for u in range(ncnk):
                            c = g0 + u
                            nc.tensor.matmul(st_ps[:, u, :], h1t_sb[:, c, :],
                                             xt_sb[:, c, :],
                                             start=True, stop=True)
                        # contiguous batches in sall: chunk c covers b=3c..
                        b0 = 32 * t + 3 * g0
                        if grp < 2:
                            nc.vector.tensor_copy(
                                out=sall_sb[:, b0:b0 + 12, :],
                                in_=st_ps[:, :, :].rearrange(
                                    "p u w -> p (u w)"),
                            )
                        else:
                            nc.vector.tensor_copy(
                                out=sall_sb[:, b0:b0 + 6, :],
                                in_=st_ps[:, :2, :].rearrange(
                                    "p u w -> p (u w)"),
                            )
                            nc.vector.tensor_copy(
                                out=sall_sb[:, b0 + 6:b0 + 8, :],
                                in_=st_ps[:, 2, :2 * F0],
                            )

                # ---- final: out2 = sum_f W1[:,f,:].T @ S^T[:, :, f] ----
                out2_ps = h1psp.tile([128, BC], F32, tag="h1_ps")
                for f in range(F0):
                    nc.tensor.matmul(out2_ps[:], w1_sb[:, f, :],
                                     sall_sb[:, :, f],
                                     start=(f == 0), stop=(f == F0 - 1))
                out2_sb = const.tile([128, BC], F32)
                nc.scalar.copy(out2_sb[:], out2_ps[:])

                nc.sync.dma_start(out=out_h.ap()[0], in_=out1_sb[:])
                nc.sync.dma_start(out=out_h.ap()[1], in_=out2_sb[:])

            if reps == 1:
                body()
            else:
                with tc.For_i(0, reps) as i:
                    body(i)

    nc.compile()
    return nc


_CACHE: dict = {}


def _get_module(reps: int = 1):
    if reps not in _CACHE:
        _CACHE[reps] = build(reps)
    return _CACHE[reps]


def run(input: np.ndarray, W0: np.ndarray, W1: np.ndarray, reps: int = 1):
    nc = _get_module(reps)
    packs = pack_weights(np.asarray(W0), np.asarray(W1))
    x_np = np.asarray(input)
    in_maps = []
    for c in range(NCORES):
        xp, xt3 = pack_x(x_np[c * BC:(c + 1) * BC])
        m = {"xp": xp, "xt3": xt3}
        m.update(packs)
        in_maps.append(m)
    res = run_bass_kernel_spmd(nc, in_maps, core_ids=list(range(NCORES)))
    out = np.empty((B, 256), dtype=np.float32)
    for c in range(NCORES):
        o = res.results[c]["out"]          # [2, 128, 256]
        out[c * BC:(c + 1) * BC, :128] = o[0].T
        out[c * BC:(c + 1) * BC, 128:] = o[1].T
    return out


def kernel(input: np.ndarray, W0: np.ndarray, W1: np.ndarray) -> np.ndarray:
    return run(input, W0, W1, reps=1)


# revision 39
# speedup vs baseline: 8.4336x; 4.1148x over previous
"""Trainium2 Bass kernel for a 2-layer CIN (Compressed Interaction Network).

Reference computation (per batch b, embedding dim d):
    h1[q] = sum_{f,g} x[f] x[g] W0[q, f*39+g]          (f,g in 0..38)
    h2[h] = sum_{f,q} x[f] h1[q] W1[h, f*128+q]        (f in 0..38, q in 0..127)
    out[b] = concat(sum_d h1, sum_d h2)                 -> [B, 256]

Device mapping (data-parallel over batch across 8 cores, 256 b's each):
  * Layer 1 uses a polarization ("sum of squares") identity so the outer
    product x (x) x never materializes:  x_i x_j = ((x_i+x_j)^2 - x_i^2 - x_j^2)/2.
    With 780 fixed linear forms V (39 singles + 741 pair sums, padded to
    896 = 7*128) and re-packed coefficients C:  h1 = C^T (V^T x)^2.
    All matmuls padded to contraction k=128 (k<128 measured 3.7x slower).
  * Layer 2 exploits  sum_d h2[b,:,d] = W1flat @ vec(S_b),
    S_b[f,q] = sum_d x[b,f,d] h1[b,q,d]  (a k=16 outer-product contraction
    per batch).  S^T is computed 3 batches at a time with one k=128 matmul
    against a host-precomputed block-diagonal transposed-x operand, after
    transposing h1 on the PE.  The final contraction is 39 k=128 matmuls.
"""

import numpy as np

import concourse.mybir as mybir
import concourse.tile as tile
from concourse import bacc
from concourse.bass import ts
from concourse.bass_utils import run_bass_kernel_spmd

B, F0, D = 2048, 39, 16
H1, H2 = 128, 128
NCORES = 8
BC = B // NCORES          # 256 batches per core
BT = 32                   # batches per tile
NT = BC // BT             # 8 tiles per core
N = BT * D                # 512 columns per tile (cols = (b, d), d inner)
NFP = 896                 # forms padded to 7*128
NCHUNK = 7
CW = 128                  # forms per chunk
GB = 3                    # batches per S-chunk (3*39=117 <= 128)
NG = 11                   # S-chunks per tile (10*3 + 1*2 = 32)
DP = 32                   # padded d-block (16 real + 16 zero)

F16 = mybir.dt.float16
F32 = mybir.dt.float32


def pack_weights(W0: np.ndarray, W1: np.ndarray):
    """Host-side repack of CIN weights into device layouts (fp16)."""
    W0m = W0[:, :, 0].reshape(H1, F0, F0).astype(np.float64)
    W1m = W1[:, :, 0].reshape(H2, F0, H1).astype(np.float64)

    V = np.zeros((128, NFP), dtype=np.float64)   # k-padded: rows 39.. = 0
    C = np.zeros((NFP, H1), dtype=np.float64)
    for i in range(F0):
        V[i, i] = 1.0
        Bi = W0m[:, i, :] + W0m[:, :, i]          # [H, F]
        C[i, :] = W0m[:, i, i] - 0.5 * (Bi.sum(axis=1) - 2.0 * W0m[:, i, i])
    k = F0
    for i in range(F0):
        for j in range(i + 1, F0):
            V[i, k] = 1.0
            V[j, k] = 1.0
            C[k, :] = 0.5 * (W0m[:, i, j] + W0m[:, j, i])
            k += 1
    c_pack = C.reshape(NCHUNK, CW, H1).transpose(1, 0, 2)   # [128, 7, 128]

    w1p = W1m.transpose(2, 1, 0)                   # [q=128, f=39, h=128]

    ident = np.eye(128, dtype=np.float16)

    return {
        "vp": np.ascontiguousarray(V, dtype=np.float16),
        "cp": np.ascontiguousarray(c_pack, dtype=np.float16),
        "w1p": np.ascontiguousarray(w1p, dtype=np.float16),
        "ident": ident,
    }


def pack_x(x_core: np.ndarray):
    """Per-core input repack: f-padded dense x + block-diagonal transposed x.

    x_core: [BC, 39, 16] float.
    Returns xp [BC, 128, 16] fp16 (f rows 39.. zero) and
    xt3 [NT, NG, 128, 117] fp16: chunk (t,c) covers batches 32t+3c+j,
    partition p=(j*32+d), col=(j*39+f), value x[b, f, d] (zero-padded).
    """
    x16 = x_core.astype(np.float16)
    xp = np.zeros((BC, 128, D), dtype=np.float16)
    xp[:, :F0, :] = x16
    # columns j-major: col = j*F0 + f so S^T lands [q, (j, f)]
    xt3 = np.zeros((NT, NG, 4, DP, GB, F0), dtype=np.float16)
    x5 = x16.reshape(NT, BT, F0, D)
    for j in range(GB):
        cmax = NG if j < GB - 1 else NG - 1
        bs = np.arange(cmax) * GB + j
        # [NT, cmax, D, F0] into block (partition j, col j)
        xt3[:, :cmax, j, :D, j, :] = x5[:, bs].transpose(0, 1, 3, 2)
    return xp, np.ascontiguousarray(
        xt3.reshape(NT, NG, 128, GB * F0))


def build(reps: int = 1, stage: str = "full"):
    """Build the per-core Bass module. reps>1 wraps the body in a HW loop
    (wall-clock timing only — the graded path uses reps=1).
    stage: cumulative subset for profiling:
      'l1' | 'h1p3' | 'trans' | 'smm' | 'nomemset' | 'full'"""
    S = ["l1", "h1p3", "trans", "smm", "full"].index(
        "full" if stage == "nomemset" else stage)
    nc = bacc.Bacc("TRN2", target_bir_lowering=False, debug=False,
                   num_devices=NCORES)

    x_h = nc.dram_tensor("xp", [BC, 128, D], F16, kind="ExternalInput")
    xt3_h = nc.dram_tensor("xt3", [NT, NG, 128, GB * F0], F16,
                           kind="ExternalInput")
    vp_h = nc.dram_tensor("vp", [128, NFP], F16, kind="ExternalInput")
    cp_h = nc.dram_tensor("cp", [CW, NCHUNK, H1], F16, kind="ExternalInput")
    w1p_h = nc.dram_tensor("w1p", [H1, F0, H2], F16, kind="ExternalInput")
    id_h = nc.dram_tensor("ident", [128, 128], F16, kind="ExternalInput")
    out_h = nc.dram_tensor("out", [2, 128, BC], F32, kind="ExternalOutput")

    x_ap = x_h.ap().rearrange("b f d -> f b d")      # [128, 256, 16]
    xt3_ap = xt3_h.ap().rearrange("t c p w -> p t c w")  # [128, 8, 11, 117]

    with tile.TileContext(nc) as tc:
        with (
            tc.tile_pool(name="const", bufs=1) as const,
            tc.tile_pool(name="xpool", bufs=2) as xpool,
            tc.tile_pool(name="xtp", bufs=2) as xtp,
            tc.tile_pool(name="ysq", bufs=2) as ysqp,
            tc.tile_pool(name="h1p3", bufs=2) as h1p3p,
            tc.tile_pool(name="h1t", bufs=2) as h1tp,
            tc.tile_pool(name="yps", bufs=2, space="PSUM") as yps,
            tc.tile_pool(name="h1ps", bufs=2, space="PSUM") as h1psp,
            tc.tile_pool(name="h1tps", bufs=2, space="PSUM") as h1tps,
            tc.tile_pool(name="stps", bufs=2, space="PSUM") as stps,
        ):
            v_sb = const.tile([128, NFP], F16)
            nc.sync.dma_start(out=v_sb[:], in_=vp_h.ap())
            c_sb = const.tile([CW, NCHUNK, H1], F16)
            nc.sync.dma_start(out=c_sb[:], in_=cp_h.ap())
            w1_sb = const.tile([H1, F0, H2], F16)
            nc.sync.dma_start(out=w1_sb[:], in_=w1p_h.ap())
            id_sb = const.tile([128, 128], F16)
            nc.sync.dma_start(out=id_sb[:], in_=id_h.ap())
            out1_sb = const.tile([128, BC], F32)
            sall_sb = const.tile([128, BC, F0], F16)   # S^T: [q, b, f]

            def body(_i=None):
                for t in range(NT):
                    x_sb = xpool.tile([128, BT, D], F16)
                    nc.sync.dma_start(out=x_sb[:], in_=x_ap[:, ts(t, BT), :])
                    x_flat = x_sb[:, :, :]           # [128, 512]
                    xt_sb = xtp.tile([128, NG, GB * F0], F16)
                    nc.sync.dma_start(out=xt_sb[:], in_=xt3_ap[:, t])

                    # ---- layer 1: h1 = C^T (V^T x)^2, all k=128 ----
                    ysq = ysqp.tile([128, NCHUNK, N], F16)
                    for j in range(NCHUNK):
                        y_ps = yps.tile([128, N], F32, tag="y")
                        nc.tensor.matmul(y_ps[:], v_sb[:, ts(j, CW)], x_flat,
                                         start=True, stop=True)
                        nc.scalar.square(ysq[:, j, :], y_ps[:])
                    h1_ps = h1psp.tile([H1, N], F32)
                    for j in range(NCHUNK):
                        nc.tensor.matmul(h1_ps[:], c_sb[:, j, :], ysq[:, j, :],
                                         start=(j == 0), stop=(j == NCHUNK - 1))
                    nc.vector.reduce_sum(
                        out=out1_sb[:, ts(t, BT)],
                        in_=h1_ps.rearrange("p (b d) -> p b d", d=D),
                        axis=mybir.AxisListType.X,
                    )

                    # ---- h1 -> padded (3b x 32d + 32z) column blocks ----
                    if S < 1:
                        continue
                    h1p3 = h1p3p.tile([128, NG, 128], F16)
                    if stage != "nomemset":
                        nc.gpsimd.memset(h1p3[:], 0.0)
                    # c = 0..9 (3 batches each), c = 10 (2 batches)
                    nc.scalar.copy(
                        h1p3[:, :10, :3 * DP].rearrange(
                            "p c (j dp) -> p c j dp", dp=DP)[:, :, :, :D],
                        h1_ps[:, :480].rearrange("p (c j d) -> p c j d",
                                                 j=GB, d=D),
                    )
                    nc.scalar.copy(
                        h1p3[:, 10, :2 * DP].rearrange(
                            "p (j dp) -> p j dp", dp=DP)[:, :, :D],
                        h1_ps[:, 480:].rearrange("p (j d) -> p j d", d=D),
                    )

                    # ---- transpose h1 blocks; S^T-matmuls (k=128) ----
                    if S < 2:
                        continue
                    h1t_sb = h1tp.tile([128, NG, 128], F16)
                    for half in range(2):            # chunks 0-5, 6-10
                        c0 = 6 * half
                        ncnk = 6 if half == 0 else 5
                        h1t_ps = h1tps.tile([128, 6, 128], F16, tag="h1t")
                        for u in range(ncnk):
                            nc.tensor.transpose(h1t_ps[:, u, :],
                                                h1p3[:, c0 + u, :], id_sb[:])
                        nc.vector.tensor_copy(
                            out=h1t_sb[:, c0:c0 + ncnk, :],
                            in_=h1t_ps[:, :ncnk, :])

                    if S < 3:
                        continue
                    for grp in range(3):             # S chunks 0-3, 4-7, 8-10
                        g0 = 4 * grp
                        ncnk = 4 if grp < 2 else 3
                        st_ps = stps.tile([128, 4, GB * F0], F32, tag="st")
                        for u in range(ncnk):
                            c = g0 + u
                            nc.tensor.matmul(st_ps[:, u, :], h1t_sb[:, c, :],
                                             xt_sb[:, c, :],
                                             start=True, stop=True)
                        # contiguous batches in sall: chunk c covers b=3c..
                        # contiguous: [q, (u, j, f)] -> sall[q, b0.., f]
                        b0 = 32 * t + 3 * g0
                        if grp < 2:
                            nc.vector.tensor_copy(
                                out=sall_sb[:, b0:b0 + 12, :],
                                in_=st_ps[:, :, :].rearrange(
                                    "p u w -> p (u w)"),
                            )
                        else:
                            nc.vector.tensor_copy(
                                out=sall_sb[:, b0:b0 + 6, :],
                                in_=st_ps[:, :2, :].rearrange(
                                    "p u w -> p (u w)"),
                            )
                            nc.vector.tensor_copy(
                                out=sall_sb[:, b0 + 6:b0 + 8, :],
                                in_=st_ps[:, 2, :2 * F0],
                            )

                # ---- final: out2 = sum_f W1[:,f,:].T @ S^T[:, :, f] ----
                if S >= 4:
                    out2_ps = h1psp.tile([128, BC], F32, tag="h1_ps")
                    for f in range(F0):
                        nc.tensor.matmul(out2_ps[:], w1_sb[:, f, :],
                                         sall_sb[:, :, f],
                                         start=(f == 0), stop=(f == F0 - 1))
                    out2_sb = const.tile([128, BC], F32)
                    nc.scalar.copy(out2_sb[:], out2_ps[:])
                    nc.sync.dma_start(out=out_h.ap()[1], in_=out2_sb[:])

                nc.sync.dma_start(out=out_h.ap()[0], in_=out1_sb[:])

            if reps == 1:
                body()
            else:
                with tc.For_i(0, reps) as i:
                    body(i)

    nc.compile()
    return nc


_CACHE: dict = {}


def _get_module(reps: int = 1):
    if reps not in _CACHE:
        _CACHE[reps] = build(reps)
    return _CACHE[reps]


def run(input: np.ndarray, W0: np.ndarray, W1: np.ndarray, reps: int = 1):
    nc = _get_module(reps)
    packs = pack_weights(np.asarray(W0), np.asarray(W1))
    x_np = np.asarray(input)
    in_maps = []
    for c in range(NCORES):
        xp, xt3 = pack_x(x_np[c * BC:(c + 1) * BC])
        m = {"xp": xp, "xt3": xt3}
        m.update(packs)
        in_maps.append(m)
    res = run_bass_kernel_spmd(nc, in_maps, core_ids=list(range(NCORES)))
    out = np.empty((B, 256), dtype=np.float32)
    for c in range(NCORES):
        o = res.results[c]["out"]          # [2, 128, 256]
        out[c * BC:(c + 1) * BC, :128] = o[0].T
        out[c * BC:(c + 1) * BC, 128:] = o[1].T
    return out


def kernel(input: np.ndarray, W0: np.ndarray, W1: np.ndarray) -> np.ndarray:
    return run(input, W0, W1, reps=1)
